# revision 8
# baseline (speedup 1.0000x reference)
"""Trainium2 Bass kernel for causal multi-head attention (12 heads, S=4096,
D=768) on 8 NeuronCores — head-sharded, zero device collectives.

Sharding: core pair (2i, 2i+1) owns heads {3i, 3i+1, 3i+2}. Core 2i runs
head A=3i in full plus query blocks [0,22) of shared head B=3i+2; core 2i+1
runs A=3i+1 plus query blocks [22,32) of B. Each core computes Q^T/K^T/V for
its two heads over all 4096 rows from the (host pre-transposed, pre-tiled)
full x, so no K/V exchange is needed. Each core emits a partial output
y_c = attn_slice @ W_out_slice; the host sums the 8 partials and folds in
all biases that are softmax-invariant or affine (K-bias drops entirely;
V-bias and out-bias become a host-side constant row).

Attention runs in scores-transposed orientation [kpos, qpos] (exp safe
without max subtraction, |s|<=8), with a ones-column per head appended to V
so the softmax denominator falls out of the PV matmul as row 64. exp'd
probabilities and V are bf16 (PE rate is identical, halves SBUF/DVE cost);
Q/K stay f32r so scores are fp32-accurate.
"""

import numpy as np

import concourse.bass as bass
import concourse.tile as tile
from concourse import bacc, mybir
from concourse.bass_utils import run_bass_kernel_spmd

F32 = mybir.dt.float32
F32R = mybir.dt.float32r
BF16 = mybir.dt.bfloat16
AF = mybir.ActivationFunctionType
ALU = mybir.AluOpType

D = 768
NH = 12
DH = 64
S = 4096
NC = 8
NEG = -1e30
B_SPLIT = 22          # shared head: even core gets q-blocks [0,22), odd [22,32)

# (q0, qw) chunk lists
A_CHUNKS = [(w * 512, 512) for w in range(8)]
B_PREFIX = [(w * 512, 512) for w in range(5)] + [(2560, 256)]
B_SUFFIX = [(2816, 512), (3328, 512), (3840, 256)]


def _emit_head(nc, hsel, chunks, qt_sb, kt_sb, v_sb, attn_sb, tri_bf, ident_bf,
               psS, psV, expp, bcp, after_chunk=None):
    """Attention for one head (hsel 0 = partitions 0:64, 1 = 64:128) over the
    given query chunks. Writes normalized head outputs into attn_sb rows
    [64*hsel, 64*hsel+64). Calls after_chunk(ci) after each chunk's emission
    so the caller can interleave out-projection work."""
    hoff = 64 * hsel
    voff = 65 * hsel
    for ci, (q0, qw) in enumerate(chunks):
        kmax = (q0 + qw) // 128
        pv = psV.tile([65, 512], F32, tag="pv")
        nb = (kmax + 1) // 2
        assert kmax % 2 == 0
        pend = []
        for b in range(nb):
            j0 = 2 * b
            sc = psS.tile([128, 1024], F32, tag="sc")
            for ji in range(2):
                j = j0 + ji
                seg = sc[:, ji * qw:(ji + 1) * qw]
                diag = q0 <= j * 128 < q0 + qw
                nc.tensor.matmul(
                    seg,
                    kt_sb[hoff:hoff + 64, j * 128:(j + 1) * 128],
                    qt_sb[hoff:hoff + 64, q0:q0 + qw],
                    start=True, stop=not diag,
                    tile_position=(hoff, 0),
                )
                if diag:
                    off = ji * qw + (j * 128 - q0)
                    nc.tensor.matmul(sc[:, off:off + 128], tri_bf[:],
                                     ident_bf[:], start=False, stop=True)
            e = expp.tile([128, 1024], BF16, tag="e")
            nc.scalar.activation(out=e[:, 0:2 * qw], in_=sc[:, 0:2 * qw],
                                 func=AF.Exp, scale=1.0)
            for ji in range(2):
                j = j0 + ji
                off = j * 128 - q0
                if off > 0:
                    nc.gpsimd.memset(
                        e[:, ji * qw:ji * qw + off].bitcast(F32), 0.0)
            # software pipeline: PV for batch b-1 was deferred until batch b's
            # scores are issued, so the PE never waits on the exp of the
            # current batch.
            pend.append((e, j0))
            if len(pend) > 1:
                _pv_batch(nc, pend.pop(0), v_sb, voff, pv, qw, kmax)
        _pv_batch(nc, pend.pop(0), v_sb, voff, pv, qw, kmax)

        # normalization: denominator row 64 -> reciprocal -> broadcast ->
        # fused multiply during psum evacuation
        rec = bcp.tile([1, 512], F32R, tag="rec")
        with nc.allow_low_precision(reason="f32r recip"):
            nc.vector.reciprocal(rec[:, 0:qw], pv[64:65, 0:qw])
        bc = bcp.tile([64, 512], F32R, tag="bc")
        nc.gpsimd.partition_broadcast(bc[:, 0:qw], rec[:, 0:qw])
        with nc.allow_low_precision(reason="f32r attn"):
            nc.vector.tensor_tensor(out=attn_sb[hoff:hoff + 64, q0:q0 + qw],
                                    in0=pv[0:64, 0:qw], in1=bc[:, 0:qw],
                                    op=ALU.mult)
        if after_chunk is not None:
            after_chunk(ci)


def _pv_batch(nc, ent, v_sb, voff, pv, qw, kmax):
    e, j0 = ent
    for ji in range(2):
        j = j0 + ji
        nc.tensor.matmul(
            pv[:, 0:qw],
            v_sb[:, j, voff:voff + 65],
            e[:, ji * qw:(ji + 1) * qw],
            start=(j == 0), stop=(j == kmax - 1),
        )


def build_program():
    nc = bacc.Bacc("TRN2", target_bir_lowering=False, debug=False,
                   num_devices=NC)

    xt = nc.dram_tensor('xt', [8, 6, 128, 512], F32, kind='ExternalInput')
    wq = nc.dram_tensor('wq', [6, 128, 128], F32, kind='ExternalInput')
    wk = nc.dram_tensor('wk', [6, 128, 128], F32, kind='ExternalInput')
    wv = nc.dram_tensor('wv', [6, 128, 128], F32, kind='ExternalInput')
    wo = nc.dram_tensor('wo', [128, D], F32, kind='ExternalInput')
    bq = nc.dram_tensor('bq', [128, 1], F32, kind='ExternalInput')
    y = nc.dram_tensor('y', [S, D], F32, kind='ExternalOutput')

    with tile.TileContext(nc) as tc:
        with tc.tile_pool(name="const", bufs=1) as const, \
             tc.tile_pool(name="proj", bufs=1) as projp, \
             tc.tile_pool(name="io", bufs=4) as iop:

            # ---------------- constants ----------------
            ident_f = const.tile([128, 128], F32)
            nc.gpsimd.memset(ident_f[:], 0.0)
            nc.gpsimd.affine_select(out=ident_f[:], in_=ident_f[:],
                                    compare_op=ALU.not_equal, fill=1.0,
                                    base=0, pattern=[[-1, 128]],
                                    channel_multiplier=1)
            ident_bf = const.tile([128, 128], BF16)
            nc.vector.tensor_copy(ident_bf[:], ident_f[:])
            scr2 = const.tile([128, 128], F32)
            nc.gpsimd.memset(scr2[:], 0.0)
            nc.gpsimd.affine_select(out=scr2[:], in_=scr2[:],
                                    compare_op=ALU.is_ge, fill=NEG,
                                    base=0, pattern=[[-1, 128]],
                                    channel_multiplier=1)
            tri_bf = const.tile([128, 128], BF16)
            nc.vector.tensor_copy(tri_bf[:], scr2[:])

            bq_sb = const.tile([128, 1], F32)
            nc.sync.dma_start(out=bq_sb[:], in_=bq[:])
            wo_sb = const.tile([128, D], F32R)
            nc.sync.dma_start(out=wo_sb[:], in_=wo[:].bitcast(F32R))

            qt_sb = projp.tile([128, S], F32R)
            kt_sb = projp.tile([128, S], F32R)
            v_sb = projp.tile([128, 32, 130], BF16)
            attn_sb = projp.tile([128, S], F32R)
            ones64_f = const.tile([128, 64], F32)
            nc.gpsimd.memset(ones64_f[:], 1.0)
            with nc.allow_low_precision(reason="bf16 ones"):
                nc.vector.tensor_copy(
                    v_sb[:].rearrange("p b (h c) -> p b h c",
                                      c=65)[:, :, :, 64:65],
                    ones64_f[:].rearrange("p (b h) -> p b h", h=2))

            # ------------- phase A: projections -------------
            with tc.tile_pool(name="xt", bufs=1) as xtp, \
                 tc.tile_pool(name="wqkv", bufs=1) as wqkvp, \
                 tc.tile_pool(name="vst", bufs=2) as vstp, \
                 tc.tile_pool(name="psP", bufs=2, space="PSUM") as psP, \
                 tc.tile_pool(name="psT", bufs=2, space="PSUM") as psT:

                xt_sb = xtp.tile([128, 8, 6, 512], F32R)
                for w in range(8):
                    nc.sync.dma_start(
                        out=xt_sb[:, w],
                        in_=xt[w].rearrange("dc p col -> p dc col").bitcast(F32R))
                w_sbs = {}
                for nm, t in (("q", wq), ("k", wk), ("v", wv)):
                    w_sb = wqkvp.tile([128, 6, 128], F32R, tag=f"w{nm}")
                    nc.sync.dma_start(
                        out=w_sb[:],
                        in_=t[:].rearrange("dc p col -> p dc col").bitcast(F32R))
                    w_sbs[nm] = w_sb

                for w in range(8):
                    ps_q = psP.tile([128, 512], F32, tag="q")
                    for dc in range(6):
                        nc.tensor.matmul(ps_q[:], w_sbs["q"][:, dc, :],
                                         xt_sb[:, w, dc, :],
                                         start=(dc == 0), stop=(dc == 5))
                    with nc.allow_low_precision(reason="f32r q"):
                        nc.vector.tensor_scalar(
                            out=qt_sb[:, w * 512:(w + 1) * 512], in0=ps_q[:],
                            scalar1=bq_sb[:, 0:1], scalar2=0.125,
                            op0=ALU.add, op1=ALU.mult)
                    ps_k = psP.tile([128, 512], F32, tag="k")
                    for dc in range(6):
                        nc.tensor.matmul(ps_k[:], w_sbs["k"][:, dc, :],
                                         xt_sb[:, w, dc, :],
                                         start=(dc == 0), stop=(dc == 5))
                    with nc.allow_low_precision(reason="f32r k"):
                        nc.vector.tensor_copy(kt_sb[:, w * 512:(w + 1) * 512],
                                              ps_k[:])
                    ps_v = psP.tile([128, 512], F32, tag="v")
                    for dc in range(6):
                        nc.tensor.matmul(ps_v[:], w_sbs["v"][:, dc, :],
                                         xt_sb[:, w, dc, :],
                                         start=(dc == 0), stop=(dc == 5))
                    vt_st = vstp.tile([128, 512], BF16, tag="vt")
                    with nc.allow_low_precision(reason="bf16 v"):
                        nc.vector.tensor_copy(vt_st[:], ps_v[:])
                    for st in range(4):
                        ps_t = psT.tile([128, 128], BF16, tag="t")
                        nc.tensor.transpose(ps_t[:],
                                            vt_st[:, st * 128:(st + 1) * 128],
                                            ident_bf[:])
                        blk = w * 4 + st
                        with nc.allow_low_precision(reason="bf16 v"):
                            nc.vector.tensor_copy(v_sb[:, blk, 0:64],
                                                  ps_t[:, 0:64])
                            nc.vector.tensor_copy(v_sb[:, blk, 65:129],
                                                  ps_t[:, 64:128])

            # ------------- phase B: attention + out-projection -------------
            with tc.tile_pool(name="exp", bufs=3) as expp, \
                 tc.tile_pool(name="bcast", bufs=2) as bcp, \
                 tc.tile_pool(name="psS", bufs=2, space="PSUM") as psS, \
                 tc.tile_pool(name="psV", bufs=2, space="PSUM") as psV, \
                 tc.tile_pool(name="psO", bufs=1, space="PSUM") as psO:

                def outproj(g):
                    ps_o = psO.tile([128, D], F32, tag="o")
                    for (n0, nw) in ((0, 512), (512, 256)):
                        nc.tensor.matmul(ps_o[:, n0:n0 + nw],
                                         attn_sb[:, g * 128:(g + 1) * 128],
                                         wo_sb[:, n0:n0 + nw],
                                         start=True, stop=True)
                    y_sb = iop.tile([128, D], F32, tag="y")
                    nc.vector.tensor_copy(y_sb[:], ps_o[:])
                    nc.sync.dma_start(out=y[g * 128:(g + 1) * 128, :],
                                      in_=y_sb[:])

                pid = nc.partition_id()
                args = (qt_sb, kt_sb, v_sb, attn_sb, tri_bf, ident_bf,
                        psS, psV, expp, bcp)
                for c in range(NC):
                    with tc.If(pid == c):
                        if c % 2 == 0:
                            nc.gpsimd.memset(
                                attn_sb[64:128, B_SPLIT * 128:S].bitcast(F32),
                                0.0)
                            _emit_head(nc, 1, B_PREFIX, *args)
                        else:
                            nc.gpsimd.memset(
                                attn_sb[64:128, 0:B_SPLIT * 128].bitcast(F32),
                                0.0)
                            _emit_head(nc, 1, B_SUFFIX, *args)

                # head A with out-projection interleaved one chunk behind
                def after_chunk(ci):
                    if ci >= 1:
                        for g in range(4 * (ci - 1), 4 * ci):
                            outproj(g)
                _emit_head(nc, 0, A_CHUNKS, *args, after_chunk=after_chunk)
                for g in range(28, 32):
                    outproj(g)

    nc.finalize()
    return nc


_CACHE = {}


def _get_program():
    if 'nc' not in _CACHE:
        _CACHE['nc'] = build_program()
    return _CACHE['nc']


def make_in_maps(x, W_qkv, b_qkv):
    """Per-core input dicts (shared host prep for kernel() and harnesses)."""
    xt = np.ascontiguousarray(
        x[0].T.reshape(6, 128, 8, 512).transpose(2, 0, 1, 3))
    in_maps = []
    for c in range(NC):
        hA = 3 * (c // 2) + (c % 2)
        hB = 3 * (c // 2) + 2
        cols = np.r_[hA * DH:(hA + 1) * DH, hB * DH:(hB + 1) * DH]
        in_maps.append({
            'xt': xt,
            'wq': np.ascontiguousarray(
                W_qkv[:, cols].reshape(6, 128, 128)),
            'wk': np.ascontiguousarray(
                W_qkv[:, D + cols].reshape(6, 128, 128)),
            'wv': np.ascontiguousarray(
                W_qkv[:, 2 * D + cols].reshape(6, 128, 128)),
            'wo': None,  # filled by caller (needs W_out)
            'bq': np.ascontiguousarray(b_qkv[cols].reshape(128, 1)),
        })
    return in_maps


def kernel(x, W_qkv, b_qkv, W_out, b_out, mask):
    x = np.asarray(x, dtype=np.float32)
    W_qkv = np.ascontiguousarray(np.asarray(W_qkv, dtype=np.float32))
    b_qkv = np.asarray(b_qkv, dtype=np.float32)
    W_out = np.ascontiguousarray(np.asarray(W_out, dtype=np.float32))
    b_out = np.asarray(b_out, dtype=np.float32)
    mask = np.asarray(mask)

    causal = np.array_equal(mask[0, 0], np.tril(np.ones((S, S), dtype=bool)))
    if not causal:
        raise NotImplementedError("only causal (tril) mask supported")

    nc = _get_program()
    in_maps = make_in_maps(x, W_qkv, b_qkv)
    for c in range(NC):
        hA = 3 * (c // 2) + (c % 2)
        hB = 3 * (c // 2) + 2
        rows = np.r_[hA * DH:(hA + 1) * DH, hB * DH:(hB + 1) * DH]
        in_maps[c]['wo'] = np.ascontiguousarray(W_out[rows, :])

    res = run_bass_kernel_spmd(nc, in_maps, list(range(NC)))

    acc = np.zeros((S, D), dtype=np.float32)
    for c in range(NC):
        acc += res.results[c]['y']
    acc += b_out + b_qkv[2 * D:3 * D] @ W_out
    return acc[None, :, :]


# revision 52
# speedup vs baseline: 1.3756x; 1.3756x over previous
"""Trainium2 Bass kernel for causal multi-head attention (12 heads, S=4096,
D=768) on 8 NeuronCores — head-sharded, zero device collectives.

Sharding: core pair (2i, 2i+1) owns heads {3i, 3i+1, 3i+2}. Core 2i runs
head A=3i in full plus query blocks [0,22) of shared head B=3i+2; core 2i+1
runs A=3i+1 plus query blocks [22,32) of B. Each core computes Q^T/K^T/V for
its two heads over all 4096 rows from the (host pre-transposed, pre-tiled)
full x, so no K/V exchange is needed. Each core emits a partial output
y_c = attn_slice @ W_out_slice; the host sums the 8 partials and folds in
all biases that are softmax-invariant or affine (K-bias drops entirely;
V-bias and out-bias become a host-side constant row).

Attention runs in scores-transposed orientation [kpos, qpos] (exp safe
without max subtraction, |s|<=8), with a ones-column per head appended to V
so the softmax denominator falls out of the PV matmul as row 64. The x / qkv
weight stream is bf16 (same PE rate as f32r, half the DMA bytes); scores
accumulate in fp32 psum, exp'd probabilities and V are bf16, attention
output and out-projection stay f32r/f32.

Schedule: the 16 head-A query chunks are driven as coroutines between the
projection window groups, so the PE pipeline stays full while the x stream
lands and the ACT engine starts exp'ing ~5us in. The back half runs inside
a per-parity branch that alternates the four biggest A chunks with head B's
chunks (two PV psum accumulators in flight) and weaves the 32 out-projection
blocks between score batches — the out-proj matmuls are exactly the filler
the PE needs to stay busy while ACT exp (the stage bottleneck) keeps pace.
"""

import numpy as np

import concourse.tile as tile
from concourse import bacc, mybir
from concourse.bass_utils import run_bass_kernel_spmd

F32 = mybir.dt.float32
F32R = mybir.dt.float32r
BF16 = mybir.dt.bfloat16
AF = mybir.ActivationFunctionType
ALU = mybir.AluOpType

D = 768
NH = 12
DH = 64
S = 4096
NC = 8
NEG = -1e30
B_SPLIT = 22          # shared head: even core gets q-blocks [0,22), odd [22,32)

A_CHUNKS = [(i * 256, 256) for i in range(16)]
B_PREFIX = [(i * 256, 256) for i in range(11)]
B_SUFFIX = [(2816 + i * 256, 256) for i in range(5)]


def _emit_chunk(nc, hsel, q0, qw, qt_sb, kt_sb, v_sb, attn_sb, tri_bf,
                ident_bf, neg_bf, psS, psV, expp, bcp, on_batch=None):
    """Generator emitting one query chunk of attention for head hsel.
    Yields once mid-way (between score batches) so the caller can interleave
    other PE work; on_batch() is invoked after each score/PV batch for
    finer-grained interleaving (out-projection blocks)."""
    hoff = 64 * hsel
    voff = 65 * hsel
    kmax = (q0 + qw) // 128
    pv = psV.tile([65, 512], F32, tag="pv")
    bpw = 1024 // qw                      # kblocks per [128,1024] psum batch
    batches = [list(range(j0, min(j0 + bpw, kmax)))
               for j0 in range(0, kmax, bpw)]
    pend = []

    def pv_batch(ent):
        e, js = ent
        for ji, j in enumerate(js):
            nc.tensor.matmul(
                pv[:, 0:qw],
                v_sb[:, j, voff:voff + 65],
                e[:, ji * qw:(ji + 1) * qw],
                start=(j == 0), stop=(j == kmax - 1),
            )

    yield_each = on_batch is not None     # branch gens: batch-level alternation
    for b, js in enumerate(batches):
        sc = psS.tile([128, 1024], F32, tag="sc")
        for ji, j in enumerate(js):
            seg = sc[:, ji * qw:(ji + 1) * qw]
            diag = q0 <= j * 128 < q0 + qw
            masked = j * 128 > q0          # leading fully-masked sub-block
            nc.tensor.matmul(
                seg,
                kt_sb[hoff:hoff + 64, j * 128:(j + 1) * 128],
                qt_sb[hoff:hoff + 64, q0:q0 + qw],
                start=True, stop=not (diag or masked),
                tile_position=(hoff, 0),
            )
            if masked:
                # all-NEG add -> exp gives exact 0; keeps the masking inside
                # the PE->ACT chain (no cross-engine memset dependency)
                off = ji * qw
                nc.tensor.matmul(sc[:, off:off + (j * 128 - q0)],
                                 neg_bf[:, 0:j * 128 - q0],
                                 ident_bf[:, 0:j * 128 - q0],
                                 start=False, stop=not diag)
            if diag:
                off = ji * qw + (j * 128 - q0)
                nc.tensor.matmul(sc[:, off:off + 128], tri_bf[:],
                                 ident_bf[:], start=False, stop=True)
        e = expp.tile([128, 1024], BF16, tag="e")
        nc.scalar.activation(out=e[:, 0:len(js) * qw],
                             in_=sc[:, 0:len(js) * qw],
                             func=AF.Exp, scale=1.0)
        pend.append((e, js))
        if len(pend) > 1:
            pv_batch(pend.pop(0))
        if on_batch is not None:
            on_batch()
        if yield_each or b == len(batches) // 2:
            yield
    pv_batch(pend.pop(0))

    rec = bcp.tile([1, 512], F32R, tag="rec")
    with nc.allow_low_precision(reason="f32r recip"):
        nc.vector.reciprocal(rec[:, 0:qw], pv[64:65, 0:qw])
    bc = bcp.tile([64, 512], F32R, tag="bc")
    nc.gpsimd.partition_broadcast(bc[:, 0:qw], rec[:, 0:qw])
    with nc.allow_low_precision(reason="f32r attn"):
        nc.vector.tensor_tensor(out=attn_sb[hoff:hoff + 64, q0:q0 + qw],
                                in0=pv[0:64, 0:qw], in1=bc[:, 0:qw],
                                op=ALU.mult)


def _exhaust(gen):
    for _ in gen:
        pass


def build_program():
    nc = bacc.Bacc("TRN2", target_bir_lowering=False, debug=False,
                   num_devices=NC)

    xt = nc.dram_tensor('xt', [8, 6, 128, 512], BF16, kind='ExternalInput')
    wq = nc.dram_tensor('wq', [6, 128, 128], BF16, kind='ExternalInput')
    wk = nc.dram_tensor('wk', [6, 128, 128], BF16, kind='ExternalInput')
    wv = nc.dram_tensor('wv', [6, 128, 128], BF16, kind='ExternalInput')
    wo = nc.dram_tensor('wo', [128, D], F32, kind='ExternalInput')
    bq = nc.dram_tensor('bq', [128, 1], F32, kind='ExternalInput')
    y = nc.dram_tensor('y', [S, D], F32, kind='ExternalOutput')

    with tile.TileContext(nc) as tc:
        with tc.tile_pool(name="const", bufs=1) as const, \
             tc.tile_pool(name="proj", bufs=1) as projp, \
             tc.tile_pool(name="io", bufs=4) as iop, \
             tc.tile_pool(name="exp", bufs=4) as expp, \
             tc.tile_pool(name="bcast", bufs=2) as bcp, \
             tc.tile_pool(name="psS", bufs=2, space="PSUM") as psS:

            # ---------------- constants ----------------
            ident_f = const.tile([128, 128], F32)
            nc.gpsimd.memset(ident_f[:], 0.0)
            nc.gpsimd.affine_select(out=ident_f[:], in_=ident_f[:],
                                    compare_op=ALU.not_equal, fill=1.0,
                                    base=0, pattern=[[-1, 128]],
                                    channel_multiplier=1)
            ident_bf = const.tile([128, 128], BF16)
            nc.vector.tensor_copy(ident_bf[:], ident_f[:])
            scr2 = const.tile([128, 128], F32)
            nc.gpsimd.memset(scr2[:], 0.0)
            nc.gpsimd.affine_select(out=scr2[:], in_=scr2[:],
                                    compare_op=ALU.is_ge, fill=NEG,
                                    base=0, pattern=[[-1, 128]],
                                    channel_multiplier=1)
            tri_bf = const.tile([128, 128], BF16)
            nc.vector.tensor_copy(tri_bf[:], scr2[:])
            nc.gpsimd.memset(scr2[:], NEG)
            neg_bf = const.tile([128, 128], BF16)
            nc.vector.tensor_copy(neg_bf[:], scr2[:])

            # weights before the x stream: the first projection matmul only
            # needs ~6us of DMA, not the whole 35us xt transfer. wo is DMA'd
            # after the xt windows (only needed ~100us in).
            bq_sb = const.tile([128, 1], F32)
            wo_sb = const.tile([128, D], F32R)

            qt_sb = projp.tile([128, S], F32R)
            kt_sb = projp.tile([128, S], F32R)
            v_sb = projp.tile([128, 32, 130], BF16)
            attn_sb = projp.tile([128, S], F32R)
            ones64_f = const.tile([128, 64], F32)
            nc.gpsimd.memset(ones64_f[:], 1.0)
            with nc.allow_low_precision(reason="bf16 ones"):
                nc.vector.tensor_copy(
                    v_sb[:].rearrange("p b (h c) -> p b h c",
                                      c=65)[:, :, :, 64:65],
                    ones64_f[:].rearrange("p (b h) -> p b h", h=2))

            def mk_args(psV):
                return (qt_sb, kt_sb, v_sb, attn_sb, tri_bf, ident_bf,
                        neg_bf, psS, psV, expp, bcp)

            # ---- part 1: projection windows ‖ head-A attention chunks ----
            with tc.tile_pool(name="xt", bufs=1) as xtp, \
                 tc.tile_pool(name="wqkv", bufs=1) as wqkvp, \
                 tc.tile_pool(name="vst", bufs=2) as vstp, \
                 tc.tile_pool(name="psP", bufs=2, space="PSUM") as psP, \
                 tc.tile_pool(name="psT", bufs=1, space="PSUM") as psT, \
                 tc.tile_pool(name="psV1", bufs=1, space="PSUM") as psV1:
                a_args = mk_args(psV1)

                w_sbs = {}
                for nm, t in (("q", wq), ("k", wk), ("v", wv)):
                    w_sbs[nm] = wqkvp.tile([128, 6, 128], BF16, tag=f"w{nm}",
                                           name=f"w{nm}_sb")

                def load_w(nm, t):
                    nc.sync.dma_start(
                        out=w_sbs[nm][:],
                        in_=t[:].rearrange("dc p col -> p dc col"))

                xt_sb = xtp.tile([128, 8, 6, 512], BF16)

                def load_xt(w, dc0=0, dc1=6):
                    nc.sync.dma_start(
                        out=xt_sb[:, w, dc0:dc1],
                        in_=xt[w, dc0:dc1].rearrange(
                            "dc p col -> p dc col"))

                # DMA issue order = need order; window 0 split in two so the
                # first matmul group starts ~1.5us sooner
                load_w("q", wq)
                load_xt(0, 0, 3)
                load_xt(0, 3, 6)
                nc.sync.dma_start(out=bq_sb[:], in_=bq[:])
                load_w("k", wk)
                load_w("v", wv)
                for w in range(1, 8):
                    load_xt(w, 0, 3)
                    load_xt(w, 3, 6)
                nc.sync.dma_start(out=wo_sb[:], in_=wo[:].bitcast(F32R))

                def proj_group(nm, w):
                    ps = psP.tile([128, 512], F32, tag="qkv")
                    for dc in range(6):
                        nc.tensor.matmul(ps[:], w_sbs[nm][:, dc, :],
                                         xt_sb[:, w, dc, :],
                                         start=(dc == 0), stop=(dc == 5))
                    if nm == "q":
                        with nc.allow_low_precision(reason="f32r q"):
                            nc.vector.tensor_scalar(
                                out=qt_sb[:, w * 512:(w + 1) * 512],
                                in0=ps[:], scalar1=bq_sb[:, 0:1],
                                scalar2=0.125, op0=ALU.add, op1=ALU.mult)
                    elif nm == "k":
                        with nc.allow_low_precision(reason="f32r k"):
                            nc.vector.tensor_copy(
                                kt_sb[:, w * 512:(w + 1) * 512], ps[:])
                    else:
                        vt_st = vstp.tile([128, 512], BF16, tag="vt")
                        with nc.allow_low_precision(reason="bf16 v"):
                            nc.vector.tensor_copy(vt_st[:], ps[:])
                        ps_t = psT.tile([128, 512], BF16, tag="t")
                        for st in range(4):
                            nc.tensor.transpose(
                                ps_t[:, st * 128:(st + 1) * 128],
                                vt_st[:, st * 128:(st + 1) * 128],
                                ident_bf[:])
                        for st in range(4):
                            blk = w * 4 + st
                            s0 = st * 128
                            with nc.allow_low_precision(reason="bf16 v"):
                                nc.vector.tensor_copy(v_sb[:, blk, 0:64],
                                                      ps_t[:, s0:s0 + 64])
                                nc.vector.tensor_copy(v_sb[:, blk, 65:129],
                                                      ps_t[:, s0 + 64:s0 + 128])

                # A-chunks 0..7 (the small, early-runnable half) are driven
                # one generator segment at a time between projection groups so
                # the PE pipeline stays full while the x stream lands
                pending = []
                _DONE = object()

                def drive_one():
                    while pending:
                        if next(pending[0], _DONE) is _DONE:
                            pending.pop(0)
                        else:
                            return

                for w in range(8):
                    proj_group("q", w)
                    drive_one()
                    proj_group("k", w)
                    drive_one()
                    proj_group("v", w)
                    if w < 4:
                        idxs = (2 * w, 2 * w + 1)
                    else:
                        idxs = (w + 4,)
                    for i in idxs:
                        q0, qw = A_CHUNKS[i]
                        pending.append(
                            _emit_chunk(nc, 0, q0, qw, *a_args))
                while pending:
                    _exhaust(pending.pop(0))

            # ---- part 2 (per-core branch): A-chunks 8..15 ‖ head B ‖
            # out-projection filler, all interleaved so the PE:ACT work ratio
            # stays balanced through the back half of the kernel ----
            with tc.tile_pool(name="psO", bufs=1, space="PSUM") as psO, \
                 tc.tile_pool(name="psV2", bufs=2, space="PSUM") as psV2:
                b_args = mk_args(psV2)

                def outproj(g):
                    ps_o = psO.tile([128, D], F32, tag="o")
                    for (n0, nw) in ((0, 512), (512, 256)):
                        nc.tensor.matmul(ps_o[:, n0:n0 + nw],
                                         attn_sb[:, g * 128:(g + 1) * 128],
                                         wo_sb[:, n0:n0 + nw],
                                         start=True, stop=True)
                    y_sb = iop.tile([128, D], F32, tag="y")
                    nc.vector.tensor_copy(y_sb[:], ps_o[:])
                    nc.sync.dma_start(out=y[g * 128:(g + 1) * 128, :],
                                      in_=y_sb[:])

                pid = nc.partition_id()
                for par in range(2):
                    with tc.If(pid % 2 == par):
                        if par == 0:
                            b_chunks = B_PREFIX
                            zero_lo, zero_hi = B_SPLIT * 128, S
                        else:
                            b_chunks = B_SUFFIX
                            zero_lo, zero_hi = 0, B_SPLIT * 128
                        nc.gpsimd.memset(
                            attn_sb[64:128, zero_lo:zero_hi].bitcast(F32), 0.0)

                        # out-projection block g is ready once head A covered
                        # it (chunk g//2 normalized) and head B covered it
                        # (chunk normalized or zero-filled)
                        a_done = [True] * 12 + [False] * 4
                        b_cov = [False] * 32
                        for g in range(zero_lo // 128, zero_hi // 128):
                            b_cov[g] = True
                        emitted = [False] * 32
                        ndone = [0]

                        def on_batch():
                            for g in range(32):
                                if (not emitted[g] and a_done[g // 2]
                                        and b_cov[g]):
                                    emitted[g] = True
                                    ndone[0] += 1
                                    outproj(g)
                                    return

                        # interleave the big A-chunks with B chunks, two
                        # generators in flight (psV2 has 2 bufs)
                        aq = list(range(12, 16))
                        bq = list(b_chunks)

                        def next_gen(kind):
                            if kind == 'a' and aq:
                                i = aq.pop(0)
                                q0, qw = A_CHUNKS[i]
                                return [kind, i,
                                        _emit_chunk(nc, 0, q0, qw, *b_args,
                                                    on_batch=on_batch)]
                            if bq:
                                q0, qw = bq.pop(0)
                                return ['b', q0,
                                        _emit_chunk(nc, 1, q0, qw, *b_args,
                                                    on_batch=on_batch)]
                            if aq:
                                i = aq.pop(0)
                                q0, qw = A_CHUNKS[i]
                                return ['a', i,
                                        _emit_chunk(nc, 0, q0, qw, *b_args,
                                                    on_batch=on_batch)]
                            return None

                        flight = [g for g in (next_gen('a'), next_gen('b'))
                                  if g is not None]
                        while flight:
                            for ent in list(flight):
                                if next(ent[2], _DONE) is _DONE:
                                    if ent[0] == 'a':
                                        a_done[ent[1]] = True
                                    else:
                                        q0 = ent[1]
                                        for g in range(q0 // 128,
                                                       q0 // 128 + 2):
                                            b_cov[g] = True
                                    flight.remove(ent)
                                    ng = next_gen(ent[0])
                                    if ng is not None:
                                        flight.append(ng)
                        for g in range(32):
                            if not emitted[g]:
                                emitted[g] = True
                                ndone[0] += 1
                                outproj(g)
                        assert ndone[0] == 32

    nc.finalize()
    return nc


_CACHE = {}


def _get_program():
    if 'nc' not in _CACHE:
        _CACHE['nc'] = build_program()
    return _CACHE['nc']


def make_in_maps(x, W_qkv, b_qkv):
    """Per-core input dicts (shared host prep for kernel() and harnesses)."""
    bf = mybir.dt.np(BF16)
    xt = np.ascontiguousarray(
        x[0].T.reshape(6, 128, 8, 512).transpose(2, 0, 1, 3)).astype(bf)
    in_maps = []
    for c in range(NC):
        hA = 3 * (c // 2) + (c % 2)
        hB = 3 * (c // 2) + 2
        cols = np.r_[hA * DH:(hA + 1) * DH, hB * DH:(hB + 1) * DH]
        in_maps.append({
            'xt': xt,
            'wq': np.ascontiguousarray(
                W_qkv[:, cols].reshape(6, 128, 128)).astype(bf),
            'wk': np.ascontiguousarray(
                W_qkv[:, D + cols].reshape(6, 128, 128)).astype(bf),
            'wv': np.ascontiguousarray(
                W_qkv[:, 2 * D + cols].reshape(6, 128, 128)).astype(bf),
            'wo': None,  # filled by caller (needs W_out)
            'bq': np.ascontiguousarray(b_qkv[cols].reshape(128, 1)),
        })
    return in_maps


def kernel(x, W_qkv, b_qkv, W_out, b_out, mask):
    x = np.asarray(x, dtype=np.float32)
    W_qkv = np.ascontiguousarray(np.asarray(W_qkv, dtype=np.float32))
    b_qkv = np.asarray(b_qkv, dtype=np.float32)
    W_out = np.ascontiguousarray(np.asarray(W_out, dtype=np.float32))
    b_out = np.asarray(b_out, dtype=np.float32)
    mask = np.asarray(mask)

    causal = np.array_equal(mask[0, 0], np.tril(np.ones((S, S), dtype=bool)))
    if not causal:
        raise NotImplementedError("only causal (tril) mask supported")

    nc = _get_program()
    in_maps = make_in_maps(x, W_qkv, b_qkv)
    for c in range(NC):
        hA = 3 * (c // 2) + (c % 2)
        hB = 3 * (c // 2) + 2
        rows = np.r_[hA * DH:(hA + 1) * DH, hB * DH:(hB + 1) * DH]
        in_maps[c]['wo'] = np.ascontiguousarray(W_out[rows, :])

    res = run_bass_kernel_spmd(nc, in_maps, list(range(NC)))

    acc = np.zeros((S, D), dtype=np.float32)
    for c in range(NC):
        acc += res.results[c]['y']
    acc += b_out + b_qkv[2 * D:3 * D] @ W_out
    return acc[None, :, :]
